# revision 23
# baseline (speedup 1.0000x reference)
"""Trainium2 Bass kernel for nn_BasicBlock (conv-SE-prune-BN residual block).

Data-parallel over batch across 8 NeuronCores, with all on-core tensors in a
128-partition pair layout: partition p = 64*(b%2) + c, free index = b//2.
Per core (B_loc = 1024 -> 512 pairs):

  stream : x is DMA'd ONCE; cast+padded into a persistent bf16 xpa buffer
           (per-pair 9x9 frames with shared zero pad rows/cols, 7-pair
           group stride 576); per-sample pooling reduced on the fly.
  conv1  : 3x3 conv as 9 tap matmuls per 7-pair group: block-diagonal
           [128,128] weights (two batch parities), dy via rhs row-slice,
           dx via shifted PSUM column windows (has_written accumulation).
  fc     : fc1-relu-fc2-sigmoid gates as two block-diagonal matmuls.
  AG     : AllGather all B*C gates; global-threshold bisection, count pass
           split ACT (sign-accum) / DVE (is_lt-accum), fp32 ones-matmul
           cross-partition total.
  P3a    : R *= relu(gate - T) with BN1 partial sums; ACT square pass.
  AR2    : AllReduce BN1 stats (parity-folded to [64,2]) -> affine coefs.
  P3b    : bn1-affine+relu into a ypad ring -> conv2 (same 9-tap scheme) ->
           BN2 partial sums on eviction + ACT square pass.
  AR3    : AllReduce BN2 stats.
  P5     : bn2-affine + residual (from bf16 xpa interior) + relu -> out.

kernel(**inputs) takes the FULL inputs and returns the FULL output.
"""
import numpy as np

import concourse.bacc as bacc
import concourse.bass as bass
import concourse.mybir as mybir
import concourse.tile as tile

F32 = mybir.dt.float32
BF16 = mybir.dt.bfloat16
I32 = mybir.dt.int32
AF = mybir.ActivationFunctionType
ALU = mybir.AluOpType
AX = mybir.AxisListType

C = 64
HW = 64
PRUNE_RATE = 0.2
EPS = 1e-5
PPG = 7            # pairs per conv group (= one PSUM bank)
PB = 81            # per-pair padded frame (9 rows x 9 cols, shared pads)
GS = PPG * PB + 9  # group stride: 7 frames + tail pad row = 576
CHUNK = 28         # pairs per stream chunk (= 4 conv groups)
BIS = 12           # bisection iterations (T to ~1.2e-4)
CA = 2400          # bisect count columns on ACT; rest counted on DVE
NB = 3             # conv groups per tap-major matmul batch
FC_B = 15          # emit fc after this conv1 batch (PE-order placement)
BIS_FROM_B = 16    # interleave bisect iterations from this conv1 batch
YSLOT = 6          # ypad ring depth (conv2 input staging)
YS = 592           # ypad slot extent (>= 9*2 + 567)


def _transpose64(nc, dst_ap, src_ap):
    for i in (0, 32):
        for j in (0, 32):
            nc.vector.transpose(out=dst_ap[j:j + 32, i:i + 32],
                                in_=src_ap[i:i + 32, j:j + 32])


def build_nc(n_cores, b_loc):
    B_glob = n_cores * b_loc
    PAIRS = b_loc // 2
    NGRP = (PAIRS + PPG - 1) // PPG
    NCHUNK = (PAIRS + CHUNK - 1) // CHUNK
    XT = (NGRP - 1) * GS + (PAIRS - (NGRP - 1) * PPG) * PB + 18
    k_prune = int(PRUNE_RATE * B_glob * C)
    D0s = float(2 * k_prune - 128 * CA)   # SA + 2*CB <= D0s <=> count <= k
    N1 = float(B_glob * HW)
    rg = [list(range(n_cores))]

    def grp_pairs(g):
        return min(PPG, PAIRS - g * PPG)

    nc = bacc.Bacc("TRN2", target_bir_lowering=False, debug=False,
                   enable_asserts=True, num_devices=n_cores)

    x_in = nc.dram_tensor("x", [b_loc, C, 8, 8], F32, kind="ExternalInput")
    w1_in = nc.dram_tensor("conv1_w", [C, C, 3, 3], F32, kind="ExternalInput")
    w2_in = nc.dram_tensor("conv2_w", [C, C, 3, 3], F32, kind="ExternalInput")
    fc1w_in = nc.dram_tensor("fc1_w", [16, C], F32, kind="ExternalInput")
    fc1b_in = nc.dram_tensor("fc1_b", [16], F32, kind="ExternalInput")
    fc2w_in = nc.dram_tensor("fc2_w", [C, 16], F32, kind="ExternalInput")
    fc2b_in = nc.dram_tensor("fc2_b", [C], F32, kind="ExternalInput")
    bn1g_in = nc.dram_tensor("bn1_g", [C], F32, kind="ExternalInput")
    bn1b_in = nc.dram_tensor("bn1_b", [C], F32, kind="ExternalInput")
    bn2g_in = nc.dram_tensor("bn2_g", [C], F32, kind="ExternalInput")
    bn2b_in = nc.dram_tensor("bn2_b", [C], F32, kind="ExternalInput")
    out_d = nc.dram_tensor("out", [b_loc, C, 8, 8], F32, kind="ExternalOutput")

    with tile.TileContext(nc) as tc:
        with (
            tc.tile_pool(name="persist", bufs=1) as pp,
            tc.tile_pool(name="small", bufs=2) as smallp,
            tc.tile_pool(name="dram", bufs=1, space="DRAM") as dramp,
        ):
            # early dummy collective absorbs cross-core start skew
            bar_sb = pp.tile([1, 1], F32, tag="bar_sb")
            bar_in = dramp.tile([1, 1], F32, tag="bar_in")
            bar_out = dramp.tile([1, 1], F32, tag="bar_out",
                                 addr_space="Shared")
            nc.vector.memset(bar_sb[:], 0)
            nc.sync.dma_start(bar_in[:], bar_sb[:])
            nc.gpsimd.collective_compute(
                "AllReduce", ALU.add, replica_groups=rg,
                ins=[bar_in.opt()], outs=[bar_out.opt()])

            # ---------------- weights / constants prep ----------------
            w1_sb = pp.tile([C, C, 3, 3], F32, tag="w1")
            w2_sb = pp.tile([C, C, 3, 3], F32, tag="w2")
            nc.sync.dma_start(w1_sb[:], w1_in[:])
            nc.sync.dma_start(w2_sb[:], w2_in[:])
            lhs1, lhs2 = {}, {}
            for (wsb, lst, nm) in ((w1_sb, lhs1, "l1"), (w2_sb, lhs2, "l2")):
                for dy in range(3):
                    for dx in range(3):
                        lt = pp.tile([128, 128], BF16, tag=f"{nm}_{dy}{dx}")
                        nc.vector.memset(lt[:], 0)
                        tp = smallp.tile([C, C], F32, tag="wtr")
                        _transpose64(nc, tp[:], wsb[:, :, dy, dx])
                        nc.vector.tensor_copy(lt[0:64, 0:64], tp[:])
                        nc.vector.tensor_copy(lt[64:128, 64:128], tp[:])
                        lst[(dy, dx)] = lt

            # fc weights, block-diagonal over batch parity; the hidden dim
            # lives at partitions 0:16 (even) / 32:48 (odd) for 32-alignment
            fc1T = pp.tile([128, 64], F32, tag="fc1T")
            fc2T = pp.tile([64, 128], F32, tag="fc2T")
            nc.vector.memset(fc1T[:], 0)
            nc.vector.memset(fc2T[:], 0)
            tmp = smallp.tile([C, C], F32, tag="fctmp")
            nc.vector.memset(tmp[:], 0)
            nc.sync.dma_start(tmp[0:16, 0:64], fc1w_in[:])
            t64 = smallp.tile([C, C], F32, tag="fct64")
            _transpose64(nc, t64[:], tmp[:])      # [64, 16] in t64[:, 0:16]
            nc.vector.tensor_copy(fc1T[0:64, 0:16], t64[:, 0:16])
            nc.vector.tensor_copy(fc1T[64:128, 32:48], t64[:, 0:16])
            tmp2 = smallp.tile([C, C], F32, tag="fctmp")
            nc.vector.memset(tmp2[:], 0)
            nc.sync.dma_start(tmp2[0:64, 0:16], fc2w_in[:])
            t64b = smallp.tile([C, C], F32, tag="fct64")
            _transpose64(nc, t64b[:], tmp2[:])    # [16, 64] in t64b[0:16, :]
            nc.vector.tensor_copy(fc2T[0:16, 0:64], t64b[0:16, :])
            nc.vector.tensor_copy(fc2T[32:48, 64:128], t64b[0:16, :])

            fc1b = pp.tile([64, 1], F32, tag="fc1b")
            nc.vector.memset(fc1b[:], 0)
            nc.sync.dma_start(fc1b[0:16, :], fc1b_in[:].unsqueeze(1))
            nc.sync.dma_start(fc1b[32:48, :], fc1b_in[:].unsqueeze(1))
            fc2b = pp.tile([128, 1], F32, tag="fc2b")
            nc.sync.dma_start(fc2b[0:64, :], fc2b_in[:].unsqueeze(1))
            nc.sync.dma_start(fc2b[64:128, :], fc2b_in[:].unsqueeze(1))

            vecs = pp.tile([C, 8], F32, tag="vecs")
            # cols: 0=bn1_g 1=bn1_b 2=bn2_g 3=bn2_b
            nc.sync.dma_start(vecs[:, 0:1], bn1g_in[:].unsqueeze(1))
            nc.sync.dma_start(vecs[:, 1:2], bn1b_in[:].unsqueeze(1))
            nc.sync.dma_start(vecs[:, 2:3], bn2g_in[:].unsqueeze(1))
            nc.sync.dma_start(vecs[:, 3:4], bn2b_in[:].unsqueeze(1))
            eps_t = pp.tile([C, 1], F32, tag="eps")
            nc.vector.memset(eps_t[:], EPS)
            ones128 = pp.tile([128, 128], F32, tag="ones")
            nc.vector.memset(ones128[:], 1.0)

            # ---------------- persistent big buffers ----------------
            xpa = pp.tile([128, XT], BF16, tag="xpa")
            R = pp.tile([128, PAIRS * HW], BF16, tag="R")
            Rq = R[:].rearrange("p (q e) -> p q e", q=PAIRS, e=HW)
            junk = pp.tile([128, n_cores * PAIRS], BF16, tag="junk")
            pooled = pp.tile([128, PAIRS], F32, tag="pooled")
            gates = pp.tile([128, PAIRS], F32, tag="gates")
            NSC = 8   # stat chunks for P3a / squares
            stats1 = pp.tile([128, 2 * NSC], F32, tag="stats1")
            stats2 = pp.tile([128, NGRP + NSC], F32, tag="stats2")
            sqf = pp.tile([128, 4], F32, tag="sqf")
            scratch = pp.tile([C, 8], F32, tag="scratch")
            cf1 = pp.tile([128, 2], F32, tag="cf1")
            cf2 = pp.tile([128, 2], F32, tag="cf2")

            # bisection state
            lh = pp.tile([128, 2], F32, tag="lh")
            Tt = pp.tile([128, 1], F32, tag="Tt")
            negT = pp.tile([128, 1], F32, tag="negT")
            cnt2 = pp.tile([128, 2], F32, tag="cnt2")
            nc.vector.memset(lh[:, 0:1], 0.0)
            nc.vector.memset(lh[:, 1:2], 1.0)

            # dram bounce buffers for collectives
            ag_in = dramp.tile([128, PAIRS], F32, tag="ag_in")
            ag_out = dramp.tile([n_cores, 128, PAIRS], F32, tag="ag_out",
                                addr_space="Shared")
            ar_in = dramp.tile([C, 2], F32, tag="ar_in")
            ar_out = dramp.tile([C, 2], F32, tag="ar_out",
                                addr_space="Shared")
            ar2_in = dramp.tile([C, 2], F32, tag="ar2_in")
            ar2_out = dramp.tile([C, 2], F32, tag="ar2_out",
                                 addr_space="Shared")

            def x_dram_ap(dram_t, p0, n):
                return dram_t[2 * p0:2 * (p0 + n)].rearrange(
                    "(i s) c h w -> (s c) i (h w)", s=2)

            def xg_interior(g, npair):
                return xpa[:, GS * g:GS * g + npair * PB].rearrange(
                    "p (q r w) -> p q r w", q=npair, r=9, w=9)[:, :, 1:9, 1:9]

            # ---------------- conv helpers ----------------
            def conv_batch(g0, ngz, lhs, src_of, pss):
                """Tap-major 9-tap matmuls over ngz groups (one PSUM bank
                each) so consecutive matmuls share the stationary weights."""
                for dy in range(3):
                    rhss = []
                    for i in range(ngz):
                        npair = grp_pairs(g0 + i)
                        off, flat = src_of(g0 + i)
                        ext = npair * PB
                        rhss.append(flat[:, off + 9 * dy:
                                         off + 9 * dy + ext].rearrange(
                            "p (a r w) -> p a r w",
                            a=npair, r=9, w=9)[:, :, 0:8, :])
                    for dx in range(3):
                        oc = 2 - dx
                        for i in range(ngz):
                            ncol = grp_pairs(g0 + i) * 72
                            nc.tensor.matmul(
                                pss[i][:, oc:oc + ncol],
                                lhs[(dy, dx)][:],
                                rhss[i],
                                start=(dy == 0 and dx == 0),
                                stop=(dy == 2 and dx == 2))

            def ps_real(ps, npair):
                return ps[:, 1:1 + npair * 72].rearrange(
                    "p (a r w) -> p a r w", a=npair, r=8, w=9)[:, :, :, 1:9]

            # ================ stream + conv1 (+fc/AG/bisect) ================
            stg_cm = tc.tile_pool(name="stgp", bufs=2)
            stgp = stg_cm.__enter__()
            gata_cm = tc.tile_pool(name="gatap", bufs=1)
            gatap = gata_cm.__enter__()
            gata = gatap.tile([128, n_cores * PAIRS], F32, tag="gata")
            GCA = n_cores * PAIRS
            psc_cm = tc.tile_pool(name="ps_conv", bufs=6, space="PSUM")
            psc = psc_cm.__enter__()
            psf_cm = tc.tile_pool(name="ps_fc", bufs=1, space="PSUM")
            psf = psf_cm.__enter__()
            psb_cm = tc.tile_pool(name="ps_bis", bufs=1, space="PSUM")
            psb = psb_cm.__enter__()

            def emit_fc():
                z1 = psf.tile([128, 512], F32, tag="zfc")
                z1s = smallp.tile([64, 512], F32, tag="z1s")
                z2 = psf.tile([128, 512], F32, tag="zfc")
                nc.tensor.matmul(z1[0:64, 0:PAIRS], fc1T[:], pooled[:],
                                 start=True, stop=True)
                nc.scalar.activation(z1s[:, 0:PAIRS], z1[0:64, 0:PAIRS],
                                     AF.Relu, scale=1.0 / HW, bias=fc1b[:])
                nc.tensor.matmul(z2[:, 0:PAIRS], fc2T[:], z1s[:, 0:PAIRS],
                                 start=True, stop=True)
                nc.scalar.activation(gates[:], z2[:, 0:PAIRS],
                                     AF.Sigmoid, bias=fc2b[:])
                nc.sync.dma_start(ag_in[:], gates[:])
                nc.gpsimd.collective_compute(
                    "AllGather", ALU.bypass, replica_groups=rg,
                    ins=[ag_in.opt()], outs=[ag_out.opt()])
                nc.sync.dma_start(
                    gata[:], ag_out[:].rearrange("n p q -> (n p q)")
                    .rearrange("(p g) -> p g", p=128))

            def bisect_iter():
                tj = smallp.tile([128, 2], F32, tag="bj")
                nc.vector.tensor_scalar(out=tj[:], in0=lh[:], scalar1=0.5,
                                        scalar2=None, op0=ALU.mult,
                                        op1=ALU.add, accum_out=Tt[:])
                # ACT half: sum of sign(T - g)
                nc.scalar.activation(junk[:, 0:CA], gata[:, 0:CA], AF.Sign,
                                     scale=-1.0, bias=Tt[:],
                                     accum_out=cnt2[:, 0:1])
                # DVE half: count of g < T
                nc.vector.tensor_scalar(out=junk[:, CA:GCA],
                                        in0=gata[:, CA:GCA],
                                        scalar1=Tt[:, 0:1], scalar2=None,
                                        op0=ALU.is_lt, op1=ALU.add,
                                        accum_out=cnt2[:, 1:2])
                # per-partition combine (SBUF), then ones-matmul reduce
                cnt1 = smallp.tile([128, 1], F32, tag="bcnt1")
                nc.vector.scalar_tensor_tensor(
                    out=cnt1[:], in0=cnt2[:, 1:2], scalar=2.0,
                    in1=cnt2[:, 0:1], op0=ALU.mult, op1=ALU.add)
                pscnt = psb.tile([128, 1], F32, tag="bps")
                nc.tensor.matmul(pscnt[:], ones128[:], cnt1[:],
                                 start=True, stop=True)
                m_le = smallp.tile([128, 1], I32, tag="bmle")
                m_gt = smallp.tile([128, 1], I32, tag="bmgt")
                nc.vector.tensor_scalar(out=m_le[:], in0=pscnt[:, 0:1],
                                        scalar1=D0s,
                                        scalar2=None, op0=ALU.is_le)
                nc.vector.tensor_scalar(out=m_gt[:], in0=pscnt[:, 0:1],
                                        scalar1=D0s,
                                        scalar2=None, op0=ALU.is_gt)
                nc.vector.copy_predicated(out=lh[:, 0:1], mask=m_le[:],
                                          data=Tt[:])
                nc.vector.copy_predicated(out=lh[:, 1:2], mask=m_gt[:],
                                          data=Tt[:])

            n_bis = [0]
            NBAT = (NGRP + NB - 1) // NB

            def emit_conv1_batch(b):
                g0 = NB * b
                ngz = min(NB, NGRP - g0)
                pss = [psc.tile([128, 512], F32, tag="cps",
                                name=f"cps_{b}_{i}") for i in range(ngz)]
                conv_batch(g0, ngz, lhs1, lambda g: (GS * g, xpa), pss)
                for i in range(ngz):
                    g = g0 + i
                    npair = grp_pairs(g)
                    nc.scalar.activation(
                        Rq[:, PPG * g:PPG * g + npair].rearrange(
                            "p q (r w) -> p q r w", r=8, w=8),
                        ps_real(pss[i], npair), AF.Copy)
                if b >= BIS_FROM_B:
                    for _ in range(2):
                        if n_bis[0] < BIS:
                            bisect_iter()
                            n_bis[0] += 1

            # stream chunks; interleave conv1 batches 0..FC_B so the PE is
            # busy while pooling completes, then fc (needs ALL pooled writes
            # emitted first), then the remaining batches with bisect iters.
            next_bat = 0
            for c in range(NCHUNK):
                p0 = c * CHUNK
                n = min(CHUNK, PAIRS - p0)
                stg = stgp.tile([128, CHUNK * HW], F32, tag="stg")
                nc.sync.dma_start(
                    stg[:, 0:n * HW].rearrange("p (i e) -> p i e", i=n),
                    x_dram_ap(x_in, p0, n))
                # zero pads + cast interiors, per covered group
                st = 0
                for g in range(4 * c, min(4 * c + 4, NGRP)):
                    npair = grp_pairs(g)
                    base = GS * g
                    fr = xpa[:, base:base + npair * PB].rearrange(
                        "p (q r w) -> p q r w", q=npair, r=9, w=9)
                    nc.gpsimd.memset(fr[:, :, 0:1, :], 0)
                    nc.gpsimd.memset(fr[:, :, 1:9, 0:1], 0)
                    tl = 18 if g == NGRP - 1 else 9
                    nc.gpsimd.memset(
                        xpa[:, base + npair * PB:base + npair * PB + tl], 0)
                    nc.scalar.activation(
                        xg_interior(g, npair),
                        stg[:, st * HW:(st + npair) * HW].rearrange(
                            "p (i h w) -> p i h w", i=npair, h=8, w=8),
                        AF.Copy)
                    st += npair
                nc.vector.tensor_reduce(
                    out=pooled[:, p0:p0 + n],
                    in_=stg[:, 0:n * HW].rearrange("p (i e) -> p i e", i=n),
                    axis=AX.X, op=ALU.add)
                while (next_bat <= FC_B
                       and PPG * NB * (next_bat + 1) <= p0 + n):
                    emit_conv1_batch(next_bat)
                    next_bat += 1
            emit_fc()
            while next_bat < NBAT:
                emit_conv1_batch(next_bat)
                next_bat += 1
            while n_bis[0] < BIS:
                bisect_iter()
                n_bis[0] += 1

            # final threshold -> -T
            tj = smallp.tile([128, 2], F32, tag="bj")
            nc.vector.tensor_scalar(out=tj[:], in0=lh[:], scalar1=0.5,
                                    scalar2=None, op0=ALU.mult,
                                    op1=ALU.add, accum_out=Tt[:])
            nc.vector.tensor_scalar(out=negT[:], in0=Tt[:], scalar1=-1.0,
                                    scalar2=None, op0=ALU.mult)
            psb_cm.__exit__(None, None, None)
            psf_cm.__exit__(None, None, None)
            psc_cm.__exit__(None, None, None)
            gata_cm.__exit__(None, None, None)
            stg_cm.__exit__(None, None, None)

            # ================ P3a: gating + BN1 partial stats ================
            nc.scalar.activation(gates[:], gates[:], AF.Relu, bias=negT[:])
            SC = (PAIRS + NSC - 1) // NSC
            for s in range(NSC):
                q0 = s * SC
                n = min(SC, PAIRS - q0)
                rsl = Rq[:, q0:q0 + n]
                sep_b = gates[:, q0:q0 + n].unsqueeze(2).broadcast_to(
                    (128, n, HW))
                nc.vector.scalar_tensor_tensor(
                    out=rsl, in0=rsl, scalar=1.0, in1=sep_b,
                    op0=ALU.mult, op1=ALU.mult,
                    accum_out=stats1[:, s:s + 1])
                nc.scalar.activation(
                    junk[:, 0:n * HW],
                    R[:, q0 * HW:(q0 + n) * HW], AF.Square,
                    accum_out=stats1[:, NSC + s:NSC + s + 1])

            def stats_allreduce(scol_ap, qcol_ap, arin, arout, cf, gcol, bcol):
                nc.vector.tensor_reduce(out=sqf[:, 0:1], in_=scol_ap,
                                        axis=AX.X, op=ALU.add)
                nc.vector.tensor_reduce(out=sqf[:, 1:2], in_=qcol_ap,
                                        axis=AX.X, op=ALU.add)
                # fold batch parities: [128,2] -> [64,2]
                fold = smallp.tile([C, 2], F32, tag="fold")
                nc.sync.dma_start(fold[:], sqf[64:128, 0:2])
                nc.vector.tensor_tensor(out=sqf[0:64, 2:4], in0=sqf[0:64, 0:2],
                                        in1=fold[:], op=ALU.add)
                nc.sync.dma_start(arin[:], sqf[0:64, 2:4])
                nc.gpsimd.collective_compute(
                    "AllReduce", ALU.add, replica_groups=rg,
                    ins=[arin.opt()], outs=[arout.opt()])
                sq_g = smallp.tile([C, 2], F32, tag="sqg")
                nc.sync.dma_start(sq_g[:], arout[:])
                # scratch cols: 0=mean 1=E[x^2] 2=-var 3=sd 4=isd
                nc.vector.tensor_scalar(out=scratch[:, 0:2], in0=sq_g[:],
                                        scalar1=1.0 / N1, scalar2=None,
                                        op0=ALU.mult)
                nc.vector.scalar_tensor_tensor(
                    out=scratch[:, 2:3], in0=scratch[:, 0:1],
                    scalar=scratch[:, 0:1], in1=scratch[:, 1:2],
                    op0=ALU.mult, op1=ALU.subtract)
                nc.scalar.activation(scratch[:, 3:4], scratch[:, 2:3],
                                     AF.Sqrt, scale=-1.0, bias=eps_t[:])
                nc.vector.reciprocal(scratch[:, 4:5], scratch[:, 3:4])
                nc.vector.tensor_tensor(out=cf[0:64, 0:1],
                                        in0=vecs[:, gcol:gcol + 1],
                                        in1=scratch[:, 4:5], op=ALU.mult)
                nc.vector.scalar_tensor_tensor(
                    out=cf[0:64, 1:2], in0=scratch[:, 0:1],
                    scalar=cf[0:64, 0:1], in1=vecs[:, bcol:bcol + 1],
                    op0=ALU.mult, op1=ALU.subtract)
                nc.vector.tensor_scalar(out=cf[0:64, 1:2], in0=cf[0:64, 1:2],
                                        scalar1=-1.0, scalar2=None,
                                        op0=ALU.mult)
                nc.sync.dma_start(cf[64:128, :], cf[0:64, :])

            stats_allreduce(stats1[:, 0:NSC], stats1[:, NSC:2 * NSC],
                            ar_in, ar_out, cf1, 0, 1)

            # ================ P3b: bn1+relu -> conv2 -> BN2 stats ============
            ypp_cm = tc.tile_pool(name="ypadp", bufs=1)
            ypp = ypp_cm.__enter__()
            ypad = ypp.tile([128, YSLOT, YS], BF16, tag="ypad")
            nc.vector.memset(ypad[:], 0)
            psc2_cm = tc.tile_pool(name="ps_conv2", bufs=6, space="PSUM")
            psc2 = psc2_cm.__enter__()
            for b in range((NGRP + NB - 1) // NB):
                g0 = NB * b
                ngz = min(NB, NGRP - g0)
                for i in range(ngz):
                    g = g0 + i
                    npair = grp_pairs(g)
                    yv = ypad[:, g % YSLOT, 0:npair * PB].rearrange(
                        "p (q r w) -> p q r w", q=npair, r=9, w=9)
                    nc.scalar.activation(
                        yv[:, :, 1:9, 1:9],
                        Rq[:, PPG * g:PPG * g + npair].rearrange(
                            "p q (r w) -> p q r w", r=8, w=8),
                        AF.Relu, scale=cf1[:, 0:1], bias=cf1[:, 1:2])
                pss = [psc2.tile([128, 512], F32, tag="cps2",
                                 name=f"cps2_{b}_{i}") for i in range(ngz)]
                conv_batch(g0, ngz, lhs2,
                           lambda g: (0, ypad[:, g % YSLOT, :]), pss)
                for i in range(ngz):
                    g = g0 + i
                    npair = grp_pairs(g)
                    nc.vector.tensor_scalar(
                        out=Rq[:, PPG * g:PPG * g + npair].rearrange(
                            "p q (r w) -> p q r w", r=8, w=8),
                        in0=ps_real(pss[i], npair), scalar1=1.0, scalar2=None,
                        op0=ALU.mult, op1=ALU.add,
                        accum_out=stats2[:, g:g + 1])
            psc2_cm.__exit__(None, None, None)
            ypp_cm.__exit__(None, None, None)
            for s in range(NSC):
                q0 = s * SC
                n = min(SC, PAIRS - q0)
                nc.scalar.activation(
                    junk[:, 0:n * HW],
                    R[:, q0 * HW:(q0 + n) * HW], AF.Square,
                    accum_out=stats2[:, NGRP + s:NGRP + s + 1])

            stats_allreduce(stats2[:, 0:NGRP], stats2[:, NGRP:NGRP + NSC],
                            ar2_in, ar2_out, cf2, 2, 3)

            # ================ P5: bn2 + residual + relu -> out ===============
            # pre = cf2a*R + cf2b on DVE; residual add from the xpa interior
            # on GpSimd (tensor_tensor never contends with DVE); relu on ACT
            pre_cm = tc.tile_pool(name="prep", bufs=3)
            prep = pre_cm.__enter__()
            GPC = 4   # groups per output chunk
            g = 0
            while g < NGRP:
                ng = min(GPC, NGRP - g)
                p0 = PPG * g
                n = sum(grp_pairs(g + i) for i in range(ng))
                pre = prep.tile([128, GPC * PPG * HW], F32, tag="pre")
                nc.vector.tensor_scalar(
                    out=pre[:, 0:n * HW], in0=R[:, p0 * HW:(p0 + n) * HW],
                    scalar1=cf2[:, 0:1], scalar2=cf2[:, 1:2],
                    op0=ALU.mult, op1=ALU.add)
                st = 0
                for i in range(ng):
                    npair = grp_pairs(g + i)
                    tt_eng = nc.vector if i < (ng + 1) // 2 else nc.gpsimd
                    tt_eng.tensor_tensor(
                        out=pre[:, st * HW:(st + npair) * HW].rearrange(
                            "p (q h w) -> p q h w", q=npair, h=8, w=8),
                        in0=pre[:, st * HW:(st + npair) * HW].rearrange(
                            "p (q h w) -> p q h w", q=npair, h=8, w=8),
                        in1=xg_interior(g + i, npair),
                        op=ALU.add)
                    st += npair
                nc.scalar.activation(pre[:, 0:n * HW], pre[:, 0:n * HW],
                                     AF.Relu)
                nc.sync.dma_start(
                    x_dram_ap(out_d, p0, n),
                    pre[:, 0:n * HW].rearrange("p (i e) -> p i e", i=n))
                g += ng
            pre_cm.__exit__(None, None, None)

    nc.compile()
    return nc


_NC_CACHE = {}


def _get_nc(n_cores, b_loc):
    key = (n_cores, b_loc)
    if key not in _NC_CACHE:
        _NC_CACHE[key] = build_nc(n_cores, b_loc)
    return _NC_CACHE[key]


def kernel(**inputs):
    from concourse.bass_utils import run_bass_kernel_spmd

    x = np.asarray(inputs["x"], dtype=np.float32)
    B = x.shape[0]
    n_cores = 8
    b_loc = B // n_cores
    nc = _get_nc(n_cores, b_loc)

    weight_names = ["conv1_w", "conv2_w", "fc1_w", "fc1_b", "fc2_w", "fc2_b",
                    "bn1_g", "bn1_b", "bn2_g", "bn2_b"]
    in_maps = []
    for c in range(n_cores):
        m = {"x": np.ascontiguousarray(x[c * b_loc:(c + 1) * b_loc])}
        for n in weight_names:
            m[n] = np.asarray(inputs[n], dtype=np.float32)
        in_maps.append(m)
    res = run_bass_kernel_spmd(nc, in_maps, core_ids=list(range(n_cores)))
    out = np.concatenate([res.results[c]["out"] for c in range(n_cores)],
                         axis=0)
    return out.astype(np.float32)


# revision 24
# speedup vs baseline: 1.0616x; 1.0616x over previous
"""Trainium2 Bass kernel for nn_BasicBlock (conv-SE-prune-BN residual block).

Data-parallel over batch across 8 NeuronCores, with all on-core tensors in a
128-partition pair layout: partition p = 64*(b%2) + c, free index = b//2.
Per core (B_loc = 1024 -> 512 pairs):

  stream : x is DMA'd ONCE; cast+padded into a persistent bf16 xpa buffer
           (per-pair 9x9 frames with shared zero pad rows/cols, 7-pair
           group stride 576); per-sample pooling reduced on the fly.
  conv1  : 3x3 conv as 9 tap matmuls per 7-pair group: block-diagonal
           [128,128] weights (two batch parities), dy via rhs row-slice,
           dx via shifted PSUM column windows (has_written accumulation).
  fc     : fc1-relu-fc2-sigmoid gates as two block-diagonal matmuls.
  AG     : AllGather all B*C gates; global-threshold bisection, count pass
           split ACT (sign-accum) / DVE (is_lt-accum), fp32 ones-matmul
           cross-partition total.
  P3a    : R *= relu(gate - T) with BN1 partial sums; ACT square pass.
  AR2    : AllReduce BN1 stats (parity-folded to [64,2]) -> affine coefs.
  P3b    : bn1-affine+relu into a ypad ring -> conv2 (same 9-tap scheme) ->
           BN2 partial sums on eviction + ACT square pass.
  AR3    : AllReduce BN2 stats.
  P5     : bn2-affine + residual (from bf16 xpa interior) + relu -> out.

kernel(**inputs) takes the FULL inputs and returns the FULL output.
"""
import numpy as np

import concourse.bacc as bacc
import concourse.bass as bass
import concourse.mybir as mybir
import concourse.tile as tile

F32 = mybir.dt.float32
BF16 = mybir.dt.bfloat16
I32 = mybir.dt.int32
AF = mybir.ActivationFunctionType
ALU = mybir.AluOpType
AX = mybir.AxisListType

C = 64
HW = 64
PRUNE_RATE = 0.2
EPS = 1e-5
PPG = 7            # pairs per conv group (= one PSUM bank)
PB = 81            # per-pair padded frame (9 rows x 9 cols, shared pads)
GS = PPG * PB + 9  # group stride: 7 frames + tail pad row = 576
CHUNK = 28         # pairs per stream chunk (= 4 conv groups)
BIS = 12           # bisection iterations (T to ~1.2e-4)
CA = 2400          # bisect count columns on ACT; rest counted on DVE
NB = 3             # conv groups per tap-major matmul batch
FC_B = 15          # emit fc after this conv1 batch (PE-order placement)
BIS_FROM_B = 16    # interleave bisect iterations from this conv1 batch
YSLOT = 6          # ypad ring depth (conv2 input staging)
YS = 592           # ypad slot extent (>= 9*2 + 567)


def _transpose64(nc, dst_ap, src_ap):
    for i in (0, 32):
        for j in (0, 32):
            nc.vector.transpose(out=dst_ap[j:j + 32, i:i + 32],
                                in_=src_ap[i:i + 32, j:j + 32])


def build_nc(n_cores, b_loc):
    B_glob = n_cores * b_loc
    PAIRS = b_loc // 2
    NGRP = (PAIRS + PPG - 1) // PPG
    NCHUNK = (PAIRS + CHUNK - 1) // CHUNK
    XT = (NGRP - 1) * GS + (PAIRS - (NGRP - 1) * PPG) * PB + 18
    k_prune = int(PRUNE_RATE * B_glob * C)
    D0s = float(2 * k_prune - 128 * CA)   # SA + 2*CB <= D0s <=> count <= k
    N1 = float(B_glob * HW)
    rg = [list(range(n_cores))]

    def grp_pairs(g):
        return min(PPG, PAIRS - g * PPG)

    nc = bacc.Bacc("TRN2", target_bir_lowering=False, debug=False,
                   enable_asserts=True, num_devices=n_cores)

    x_in = nc.dram_tensor("x", [b_loc, C, 8, 8], F32, kind="ExternalInput")
    w1_in = nc.dram_tensor("conv1_w", [C, C, 3, 3], F32, kind="ExternalInput")
    w2_in = nc.dram_tensor("conv2_w", [C, C, 3, 3], F32, kind="ExternalInput")
    fc1w_in = nc.dram_tensor("fc1_w", [16, C], F32, kind="ExternalInput")
    fc1b_in = nc.dram_tensor("fc1_b", [16], F32, kind="ExternalInput")
    fc2w_in = nc.dram_tensor("fc2_w", [C, 16], F32, kind="ExternalInput")
    fc2b_in = nc.dram_tensor("fc2_b", [C], F32, kind="ExternalInput")
    bn1g_in = nc.dram_tensor("bn1_g", [C], F32, kind="ExternalInput")
    bn1b_in = nc.dram_tensor("bn1_b", [C], F32, kind="ExternalInput")
    bn2g_in = nc.dram_tensor("bn2_g", [C], F32, kind="ExternalInput")
    bn2b_in = nc.dram_tensor("bn2_b", [C], F32, kind="ExternalInput")
    out_d = nc.dram_tensor("out", [b_loc, C, 8, 8], F32, kind="ExternalOutput")

    with tile.TileContext(nc) as tc:
        with (
            tc.tile_pool(name="persist", bufs=1) as pp,
            tc.tile_pool(name="small", bufs=2) as smallp,
            tc.tile_pool(name="dram", bufs=1, space="DRAM") as dramp,
        ):
            # early dummy collective absorbs cross-core start skew
            bar_sb = pp.tile([1, 1], F32, tag="bar_sb")
            bar_in = dramp.tile([1, 1], F32, tag="bar_in")
            bar_out = dramp.tile([1, 1], F32, tag="bar_out",
                                 addr_space="Shared")
            nc.vector.memset(bar_sb[:], 0)
            nc.sync.dma_start(bar_in[:], bar_sb[:])
            nc.gpsimd.collective_compute(
                "AllReduce", ALU.add, replica_groups=rg,
                ins=[bar_in.opt()], outs=[bar_out.opt()])

            # ---------------- weights / constants prep ----------------
            w1_sb = pp.tile([C, C, 3, 3], F32, tag="w1")
            w2_sb = pp.tile([C, C, 3, 3], F32, tag="w2")
            nc.sync.dma_start(w1_sb[:], w1_in[:])
            nc.sync.dma_start(w2_sb[:], w2_in[:])
            lhs1, lhs2 = {}, {}
            for (wsb, lst, nm) in ((w1_sb, lhs1, "l1"), (w2_sb, lhs2, "l2")):
                for dy in range(3):
                    for dx in range(3):
                        lt = pp.tile([128, 128], BF16, tag=f"{nm}_{dy}{dx}")
                        nc.vector.memset(lt[:], 0)
                        tp = smallp.tile([C, C], F32, tag="wtr")
                        _transpose64(nc, tp[:], wsb[:, :, dy, dx])
                        nc.vector.tensor_copy(lt[0:64, 0:64], tp[:])
                        nc.vector.tensor_copy(lt[64:128, 64:128], tp[:])
                        lst[(dy, dx)] = lt

            # fc weights, block-diagonal over batch parity; the hidden dim
            # lives at partitions 0:16 (even) / 32:48 (odd) for 32-alignment
            fc1T = pp.tile([128, 64], F32, tag="fc1T")
            fc2T = pp.tile([64, 128], F32, tag="fc2T")
            nc.vector.memset(fc1T[:], 0)
            nc.vector.memset(fc2T[:], 0)
            tmp = smallp.tile([C, C], F32, tag="fctmp")
            nc.vector.memset(tmp[:], 0)
            nc.sync.dma_start(tmp[0:16, 0:64], fc1w_in[:])
            t64 = smallp.tile([C, C], F32, tag="fct64")
            _transpose64(nc, t64[:], tmp[:])      # [64, 16] in t64[:, 0:16]
            nc.vector.tensor_copy(fc1T[0:64, 0:16], t64[:, 0:16])
            nc.vector.tensor_copy(fc1T[64:128, 32:48], t64[:, 0:16])
            tmp2 = smallp.tile([C, C], F32, tag="fctmp")
            nc.vector.memset(tmp2[:], 0)
            nc.sync.dma_start(tmp2[0:64, 0:16], fc2w_in[:])
            t64b = smallp.tile([C, C], F32, tag="fct64")
            _transpose64(nc, t64b[:], tmp2[:])    # [16, 64] in t64b[0:16, :]
            nc.vector.tensor_copy(fc2T[0:16, 0:64], t64b[0:16, :])
            nc.vector.tensor_copy(fc2T[32:48, 64:128], t64b[0:16, :])

            fc1b = pp.tile([64, 1], F32, tag="fc1b")
            nc.vector.memset(fc1b[:], 0)
            nc.sync.dma_start(fc1b[0:16, :], fc1b_in[:].unsqueeze(1))
            nc.sync.dma_start(fc1b[32:48, :], fc1b_in[:].unsqueeze(1))
            fc2b = pp.tile([128, 1], F32, tag="fc2b")
            nc.sync.dma_start(fc2b[0:64, :], fc2b_in[:].unsqueeze(1))
            nc.sync.dma_start(fc2b[64:128, :], fc2b_in[:].unsqueeze(1))

            vecs = pp.tile([C, 8], F32, tag="vecs")
            # cols: 0=bn1_g 1=bn1_b 2=bn2_g 3=bn2_b
            nc.sync.dma_start(vecs[:, 0:1], bn1g_in[:].unsqueeze(1))
            nc.sync.dma_start(vecs[:, 1:2], bn1b_in[:].unsqueeze(1))
            nc.sync.dma_start(vecs[:, 2:3], bn2g_in[:].unsqueeze(1))
            nc.sync.dma_start(vecs[:, 3:4], bn2b_in[:].unsqueeze(1))
            eps_t = pp.tile([C, 1], F32, tag="eps")
            nc.vector.memset(eps_t[:], EPS)
            ones128 = pp.tile([128, 128], F32, tag="ones")
            nc.vector.memset(ones128[:], 1.0)

            # ---------------- persistent big buffers ----------------
            xpa = pp.tile([128, XT], BF16, tag="xpa")
            R = pp.tile([128, PAIRS * HW], BF16, tag="R")
            Rq = R[:].rearrange("p (q e) -> p q e", q=PAIRS, e=HW)
            junk = pp.tile([128, n_cores * PAIRS], BF16, tag="junk")
            pooled = pp.tile([128, PAIRS], F32, tag="pooled")
            gates = pp.tile([128, PAIRS], F32, tag="gates")
            NSC = 8   # stat chunks for P3a / squares
            stats1 = pp.tile([128, 2 * NSC], F32, tag="stats1")
            stats2 = pp.tile([128, NGRP + NSC], F32, tag="stats2")
            sqf = pp.tile([128, 4], F32, tag="sqf")
            scratch = pp.tile([C, 8], F32, tag="scratch")
            cf1 = pp.tile([128, 2], F32, tag="cf1")
            cf2 = pp.tile([128, 2], F32, tag="cf2")

            # bisection state
            lh = pp.tile([128, 2], F32, tag="lh")
            Tt = pp.tile([128, 1], F32, tag="Tt")
            negT = pp.tile([128, 1], F32, tag="negT")
            cnt2 = pp.tile([128, 2], F32, tag="cnt2")
            nc.vector.memset(lh[:, 0:1], 0.0)
            nc.vector.memset(lh[:, 1:2], 1.0)

            # dram bounce buffers for collectives
            ag_in = dramp.tile([128, PAIRS], F32, tag="ag_in")
            ag_out = dramp.tile([n_cores, 128, PAIRS], F32, tag="ag_out",
                                addr_space="Shared")
            ar_in = dramp.tile([C, 2], F32, tag="ar_in")
            ar_out = dramp.tile([C, 2], F32, tag="ar_out",
                                addr_space="Shared")
            ar2_in = dramp.tile([C, 2], F32, tag="ar2_in")
            ar2_out = dramp.tile([C, 2], F32, tag="ar2_out",
                                 addr_space="Shared")

            def x_dram_ap(dram_t, p0, n):
                return dram_t[2 * p0:2 * (p0 + n)].rearrange(
                    "(i s) c h w -> (s c) i (h w)", s=2)

            def xg_interior(g, npair):
                return xpa[:, GS * g:GS * g + npair * PB].rearrange(
                    "p (q r w) -> p q r w", q=npair, r=9, w=9)[:, :, 1:9, 1:9]

            # ---------------- conv helpers ----------------
            def conv_batch(g0, ngz, lhs, src_of, pss):
                """Tap-major 9-tap matmuls over ngz groups (one PSUM bank
                each) so consecutive matmuls share the stationary weights."""
                for dy in range(3):
                    rhss = []
                    for i in range(ngz):
                        npair = grp_pairs(g0 + i)
                        off, flat = src_of(g0 + i)
                        ext = npair * PB
                        rhss.append(flat[:, off + 9 * dy:
                                         off + 9 * dy + ext].rearrange(
                            "p (a r w) -> p a r w",
                            a=npair, r=9, w=9)[:, :, 0:8, :])
                    for dx in range(3):
                        oc = 2 - dx
                        for i in range(ngz):
                            ncol = grp_pairs(g0 + i) * 72
                            nc.tensor.matmul(
                                pss[i][:, oc:oc + ncol],
                                lhs[(dy, dx)][:],
                                rhss[i],
                                start=(dy == 0 and dx == 0),
                                stop=(dy == 2 and dx == 2))

            def ps_real(ps, npair):
                return ps[:, 1:1 + npair * 72].rearrange(
                    "p (a r w) -> p a r w", a=npair, r=8, w=9)[:, :, :, 1:9]

            # ================ stream + conv1 (+fc/AG/bisect) ================
            stg_cm = tc.tile_pool(name="stgp", bufs=2)
            stgp = stg_cm.__enter__()
            gata_cm = tc.tile_pool(name="gatap", bufs=1)
            gatap = gata_cm.__enter__()
            gata = gatap.tile([128, n_cores * PAIRS], F32, tag="gata")
            GCA = n_cores * PAIRS
            psc_cm = tc.tile_pool(name="ps_conv", bufs=6, space="PSUM")
            psc = psc_cm.__enter__()
            psf_cm = tc.tile_pool(name="ps_fc", bufs=1, space="PSUM")
            psf = psf_cm.__enter__()
            psb_cm = tc.tile_pool(name="ps_bis", bufs=1, space="PSUM")
            psb = psb_cm.__enter__()

            def emit_fc():
                z1 = psf.tile([128, 512], F32, tag="zfc")
                z1s = smallp.tile([64, 512], F32, tag="z1s")
                z2 = psf.tile([128, 512], F32, tag="zfc")
                nc.tensor.matmul(z1[0:64, 0:PAIRS], fc1T[:], pooled[:],
                                 start=True, stop=True)
                nc.scalar.activation(z1s[:, 0:PAIRS], z1[0:64, 0:PAIRS],
                                     AF.Relu, scale=1.0 / HW, bias=fc1b[:])
                nc.tensor.matmul(z2[:, 0:PAIRS], fc2T[:], z1s[:, 0:PAIRS],
                                 start=True, stop=True)
                nc.scalar.activation(gates[:], z2[:, 0:PAIRS],
                                     AF.Sigmoid, bias=fc2b[:])
                nc.sync.dma_start(ag_in[:], gates[:])
                nc.gpsimd.collective_compute(
                    "AllGather", ALU.bypass, replica_groups=rg,
                    ins=[ag_in.opt()], outs=[ag_out.opt()])
                nc.sync.dma_start(
                    gata[:], ag_out[:].rearrange("n p q -> (n p q)")
                    .rearrange("(p g) -> p g", p=128))

            def bisect_iter():
                tj = smallp.tile([128, 2], F32, tag="bj")
                nc.vector.tensor_scalar(out=tj[:], in0=lh[:], scalar1=0.5,
                                        scalar2=None, op0=ALU.mult,
                                        op1=ALU.add, accum_out=Tt[:])
                # ACT half: sum of sign(T - g)
                nc.scalar.activation(junk[:, 0:CA], gata[:, 0:CA], AF.Sign,
                                     scale=-1.0, bias=Tt[:],
                                     accum_out=cnt2[:, 0:1])
                # DVE half: count of g < T
                nc.vector.tensor_scalar(out=junk[:, CA:GCA],
                                        in0=gata[:, CA:GCA],
                                        scalar1=Tt[:, 0:1], scalar2=None,
                                        op0=ALU.is_lt, op1=ALU.add,
                                        accum_out=cnt2[:, 1:2])
                # per-partition combine (SBUF), then ones-matmul reduce
                cnt1 = smallp.tile([128, 1], F32, tag="bcnt1")
                nc.vector.scalar_tensor_tensor(
                    out=cnt1[:], in0=cnt2[:, 1:2], scalar=2.0,
                    in1=cnt2[:, 0:1], op0=ALU.mult, op1=ALU.add)
                pscnt = psb.tile([128, 1], F32, tag="bps")
                nc.tensor.matmul(pscnt[:], ones128[:], cnt1[:],
                                 start=True, stop=True)
                m_le = smallp.tile([128, 1], I32, tag="bmle")
                m_gt = smallp.tile([128, 1], I32, tag="bmgt")
                nc.vector.tensor_scalar(out=m_le[:], in0=pscnt[:, 0:1],
                                        scalar1=D0s,
                                        scalar2=None, op0=ALU.is_le)
                nc.vector.tensor_scalar(out=m_gt[:], in0=pscnt[:, 0:1],
                                        scalar1=D0s,
                                        scalar2=None, op0=ALU.is_gt)
                nc.vector.copy_predicated(out=lh[:, 0:1], mask=m_le[:],
                                          data=Tt[:])
                nc.vector.copy_predicated(out=lh[:, 1:2], mask=m_gt[:],
                                          data=Tt[:])

            n_bis = [0]
            NBAT = (NGRP + NB - 1) // NB

            def emit_conv1_batch(b):
                g0 = NB * b
                ngz = min(NB, NGRP - g0)
                pss = [psc.tile([128, 512], F32, tag="cps",
                                name=f"cps_{b}_{i}") for i in range(ngz)]
                conv_batch(g0, ngz, lhs1, lambda g: (GS * g, xpa), pss)
                for i in range(ngz):
                    g = g0 + i
                    npair = grp_pairs(g)
                    nc.scalar.activation(
                        Rq[:, PPG * g:PPG * g + npair].rearrange(
                            "p q (r w) -> p q r w", r=8, w=8),
                        ps_real(pss[i], npair), AF.Copy)
                if b >= BIS_FROM_B:
                    for _ in range(2):
                        if n_bis[0] < BIS:
                            bisect_iter()
                            n_bis[0] += 1

            # stream chunks; interleave conv1 batches 0..FC_B so the PE is
            # busy while pooling completes, then fc (needs ALL pooled writes
            # emitted first), then the remaining batches with bisect iters.
            next_bat = 0
            for c in range(NCHUNK):
                p0 = c * CHUNK
                n = min(CHUNK, PAIRS - p0)
                stg = stgp.tile([128, CHUNK * HW], F32, tag="stg")
                nc.sync.dma_start(
                    stg[:, 0:n * HW].rearrange("p (i e) -> p i e", i=n),
                    x_dram_ap(x_in, p0, n))
                # zero pads + cast interiors, per covered group
                st = 0
                for g in range(4 * c, min(4 * c + 4, NGRP)):
                    npair = grp_pairs(g)
                    base = GS * g
                    fr = xpa[:, base:base + npair * PB].rearrange(
                        "p (q r w) -> p q r w", q=npair, r=9, w=9)
                    nc.vector.memset(fr[:, :, 0:1, :], 0)
                    nc.vector.memset(fr[:, :, 1:9, 0:1], 0)
                    tl = 18 if g == NGRP - 1 else 9
                    nc.vector.memset(
                        xpa[:, base + npair * PB:base + npair * PB + tl], 0)
                    nc.scalar.activation(
                        xg_interior(g, npair),
                        stg[:, st * HW:(st + npair) * HW].rearrange(
                            "p (i h w) -> p i h w", i=npair, h=8, w=8),
                        AF.Copy)
                    st += npair
                nc.vector.tensor_reduce(
                    out=pooled[:, p0:p0 + n],
                    in_=stg[:, 0:n * HW].rearrange("p (i e) -> p i e", i=n),
                    axis=AX.X, op=ALU.add)
                while (next_bat <= FC_B
                       and PPG * NB * (next_bat + 1) <= p0 + n):
                    emit_conv1_batch(next_bat)
                    next_bat += 1
            emit_fc()
            while next_bat < NBAT:
                emit_conv1_batch(next_bat)
                next_bat += 1
            while n_bis[0] < BIS:
                bisect_iter()
                n_bis[0] += 1

            # final threshold -> -T
            tj = smallp.tile([128, 2], F32, tag="bj")
            nc.vector.tensor_scalar(out=tj[:], in0=lh[:], scalar1=0.5,
                                    scalar2=None, op0=ALU.mult,
                                    op1=ALU.add, accum_out=Tt[:])
            nc.vector.tensor_scalar(out=negT[:], in0=Tt[:], scalar1=-1.0,
                                    scalar2=None, op0=ALU.mult)
            psb_cm.__exit__(None, None, None)
            psf_cm.__exit__(None, None, None)
            psc_cm.__exit__(None, None, None)
            gata_cm.__exit__(None, None, None)
            stg_cm.__exit__(None, None, None)

            # ================ P3a: gating + BN1 partial stats ================
            nc.scalar.activation(gates[:], gates[:], AF.Relu, bias=negT[:])
            SC = (PAIRS + NSC - 1) // NSC
            for s in range(NSC):
                q0 = s * SC
                n = min(SC, PAIRS - q0)
                rsl = Rq[:, q0:q0 + n]
                sep_b = gates[:, q0:q0 + n].unsqueeze(2).broadcast_to(
                    (128, n, HW))
                nc.vector.scalar_tensor_tensor(
                    out=rsl, in0=rsl, scalar=1.0, in1=sep_b,
                    op0=ALU.mult, op1=ALU.mult,
                    accum_out=stats1[:, s:s + 1])
                nc.scalar.activation(
                    junk[:, 0:n * HW],
                    R[:, q0 * HW:(q0 + n) * HW], AF.Square,
                    accum_out=stats1[:, NSC + s:NSC + s + 1])

            def stats_allreduce(scol_ap, qcol_ap, arin, arout, cf, gcol, bcol):
                nc.vector.tensor_reduce(out=sqf[:, 0:1], in_=scol_ap,
                                        axis=AX.X, op=ALU.add)
                nc.vector.tensor_reduce(out=sqf[:, 1:2], in_=qcol_ap,
                                        axis=AX.X, op=ALU.add)
                # fold batch parities: [128,2] -> [64,2]
                fold = smallp.tile([C, 2], F32, tag="fold")
                nc.sync.dma_start(fold[:], sqf[64:128, 0:2])
                nc.vector.tensor_tensor(out=sqf[0:64, 2:4], in0=sqf[0:64, 0:2],
                                        in1=fold[:], op=ALU.add)
                nc.sync.dma_start(arin[:], sqf[0:64, 2:4])
                nc.gpsimd.collective_compute(
                    "AllReduce", ALU.add, replica_groups=rg,
                    ins=[arin.opt()], outs=[arout.opt()])
                sq_g = smallp.tile([C, 2], F32, tag="sqg")
                nc.sync.dma_start(sq_g[:], arout[:])
                # scratch cols: 0=mean 1=E[x^2] 2=-var 3=sd 4=isd
                nc.vector.tensor_scalar(out=scratch[:, 0:2], in0=sq_g[:],
                                        scalar1=1.0 / N1, scalar2=None,
                                        op0=ALU.mult)
                nc.vector.scalar_tensor_tensor(
                    out=scratch[:, 2:3], in0=scratch[:, 0:1],
                    scalar=scratch[:, 0:1], in1=scratch[:, 1:2],
                    op0=ALU.mult, op1=ALU.subtract)
                nc.scalar.activation(scratch[:, 3:4], scratch[:, 2:3],
                                     AF.Sqrt, scale=-1.0, bias=eps_t[:])
                nc.vector.reciprocal(scratch[:, 4:5], scratch[:, 3:4])
                nc.vector.tensor_tensor(out=cf[0:64, 0:1],
                                        in0=vecs[:, gcol:gcol + 1],
                                        in1=scratch[:, 4:5], op=ALU.mult)
                nc.vector.scalar_tensor_tensor(
                    out=cf[0:64, 1:2], in0=scratch[:, 0:1],
                    scalar=cf[0:64, 0:1], in1=vecs[:, bcol:bcol + 1],
                    op0=ALU.mult, op1=ALU.subtract)
                nc.vector.tensor_scalar(out=cf[0:64, 1:2], in0=cf[0:64, 1:2],
                                        scalar1=-1.0, scalar2=None,
                                        op0=ALU.mult)
                nc.sync.dma_start(cf[64:128, :], cf[0:64, :])

            stats_allreduce(stats1[:, 0:NSC], stats1[:, NSC:2 * NSC],
                            ar_in, ar_out, cf1, 0, 1)

            # ================ P3b: bn1+relu -> conv2 -> BN2 stats ============
            ypp_cm = tc.tile_pool(name="ypadp", bufs=1)
            ypp = ypp_cm.__enter__()
            ypad = ypp.tile([128, YSLOT, YS], BF16, tag="ypad")
            nc.vector.memset(ypad[:], 0)
            psc2_cm = tc.tile_pool(name="ps_conv2", bufs=6, space="PSUM")
            psc2 = psc2_cm.__enter__()
            for b in range((NGRP + NB - 1) // NB):
                g0 = NB * b
                ngz = min(NB, NGRP - g0)
                for i in range(ngz):
                    g = g0 + i
                    npair = grp_pairs(g)
                    yv = ypad[:, g % YSLOT, 0:npair * PB].rearrange(
                        "p (q r w) -> p q r w", q=npair, r=9, w=9)
                    nc.scalar.activation(
                        yv[:, :, 1:9, 1:9],
                        Rq[:, PPG * g:PPG * g + npair].rearrange(
                            "p q (r w) -> p q r w", r=8, w=8),
                        AF.Relu, scale=cf1[:, 0:1], bias=cf1[:, 1:2])
                pss = [psc2.tile([128, 512], F32, tag="cps2",
                                 name=f"cps2_{b}_{i}") for i in range(ngz)]
                conv_batch(g0, ngz, lhs2,
                           lambda g: (0, ypad[:, g % YSLOT, :]), pss)
                for i in range(ngz):
                    g = g0 + i
                    npair = grp_pairs(g)
                    nc.vector.tensor_scalar(
                        out=Rq[:, PPG * g:PPG * g + npair].rearrange(
                            "p q (r w) -> p q r w", r=8, w=8),
                        in0=ps_real(pss[i], npair), scalar1=1.0, scalar2=None,
                        op0=ALU.mult, op1=ALU.add,
                        accum_out=stats2[:, g:g + 1])
            psc2_cm.__exit__(None, None, None)
            ypp_cm.__exit__(None, None, None)
            for s in range(NSC):
                q0 = s * SC
                n = min(SC, PAIRS - q0)
                nc.scalar.activation(
                    junk[:, 0:n * HW],
                    R[:, q0 * HW:(q0 + n) * HW], AF.Square,
                    accum_out=stats2[:, NGRP + s:NGRP + s + 1])

            stats_allreduce(stats2[:, 0:NGRP], stats2[:, NGRP:NGRP + NSC],
                            ar2_in, ar2_out, cf2, 2, 3)

            # ================ P5: bn2 + residual + relu -> out ===============
            # pre = cf2a*R + cf2b on DVE; residual add from the xpa interior
            # on GpSimd (tensor_tensor never contends with DVE); relu on ACT
            pre_cm = tc.tile_pool(name="prep", bufs=3)
            prep = pre_cm.__enter__()
            GPC = 4   # groups per output chunk
            g = 0
            while g < NGRP:
                ng = min(GPC, NGRP - g)
                p0 = PPG * g
                n = sum(grp_pairs(g + i) for i in range(ng))
                pre = prep.tile([128, GPC * PPG * HW], F32, tag="pre")
                nc.vector.tensor_scalar(
                    out=pre[:, 0:n * HW], in0=R[:, p0 * HW:(p0 + n) * HW],
                    scalar1=cf2[:, 0:1], scalar2=cf2[:, 1:2],
                    op0=ALU.mult, op1=ALU.add)
                st = 0
                for i in range(ng):
                    npair = grp_pairs(g + i)
                    tt_eng = nc.vector if i < (ng + 1) // 2 else nc.gpsimd
                    tt_eng.tensor_tensor(
                        out=pre[:, st * HW:(st + npair) * HW].rearrange(
                            "p (q h w) -> p q h w", q=npair, h=8, w=8),
                        in0=pre[:, st * HW:(st + npair) * HW].rearrange(
                            "p (q h w) -> p q h w", q=npair, h=8, w=8),
                        in1=xg_interior(g + i, npair),
                        op=ALU.add)
                    st += npair
                nc.scalar.activation(pre[:, 0:n * HW], pre[:, 0:n * HW],
                                     AF.Relu)
                nc.sync.dma_start(
                    x_dram_ap(out_d, p0, n),
                    pre[:, 0:n * HW].rearrange("p (i e) -> p i e", i=n))
                g += ng
            pre_cm.__exit__(None, None, None)

    nc.compile()
    return nc


_NC_CACHE = {}


def _get_nc(n_cores, b_loc):
    key = (n_cores, b_loc)
    if key not in _NC_CACHE:
        _NC_CACHE[key] = build_nc(n_cores, b_loc)
    return _NC_CACHE[key]


def kernel(**inputs):
    from concourse.bass_utils import run_bass_kernel_spmd

    x = np.asarray(inputs["x"], dtype=np.float32)
    B = x.shape[0]
    n_cores = 8
    b_loc = B // n_cores
    nc = _get_nc(n_cores, b_loc)

    weight_names = ["conv1_w", "conv2_w", "fc1_w", "fc1_b", "fc2_w", "fc2_b",
                    "bn1_g", "bn1_b", "bn2_g", "bn2_b"]
    in_maps = []
    for c in range(n_cores):
        m = {"x": np.ascontiguousarray(x[c * b_loc:(c + 1) * b_loc])}
        for n in weight_names:
            m[n] = np.asarray(inputs[n], dtype=np.float32)
        in_maps.append(m)
    res = run_bass_kernel_spmd(nc, in_maps, core_ids=list(range(n_cores)))
    out = np.concatenate([res.results[c]["out"] for c in range(n_cores)],
                         axis=0)
    return out.astype(np.float32)
